# revision 21
# baseline (speedup 1.0000x reference)
"""Fused EmbeddingBag(mean) + Linear kernel for Trainium2, 8-core data-parallel.

Strategy: batch is sharded 8 ways (2048 bags/core). Each core processes 16
tiles of 128 bags. The gather uses the dedicated SWDGE dma_gather
(InstDMAGatherAnt) instruction, spread round-robin over all 4 SWDGE queues —
descriptor generation parallelizes almost linearly across queues (~8.5ns/desc
per queue, ~2.2ns/desc with 4).

dma_gather indices are int16, so they can only address a 32768-row window of
the table, and the SWDGE ring limits one gather to 1024 indices. The table is
rebuilt on the host with a zero row interleaved every 32767 vocab rows
(new_id = id + id//32767 + 1), giving four windows of 32768 rows whose first
row is all-zero. Per 128-bag tile and per window, the bags' tokens belonging
to that window are packed into columns (bag = partition = stream position %
128), padded with window-relative index 0 (the zero row), so invalid/missing
slots gather zeros and a plain column sum equals the masked sum. Bags are
sorted by length per core so tiles have homogeneous lengths and the per-tile
column budgets (computed from the actual data at build time, maxed across
cores so all cores share one program) stay small.

Each <=8-column gather chunk lands in its own small buffer from a deep pool
and is immediately reduced (strided-AP vector reduce) and accumulated into the
tile's sum — fine-grained consumption keeps all 4 SWDGE queues busy. All
indices and per-bag aux values load in two upfront DMAs on the sync HWDGE
queue. A single matmul against [W.T; b; null_embedding] applies projection,
bias, and the empty-bag null-embedding select in one shot (per-bag scale
1/max(len,1) and the two select flags are host-precomputed). The host
un-permutes the sorted outputs at the end.
"""

import sys

sys.path.insert(0, "/opt/trn_rl_repo")

from contextlib import ExitStack

import numpy as np

import concourse.bass as bass
import concourse.bacc as bacc
import concourse.mybir as mybir
import concourse.tile as tile
from concourse.masks import make_identity

VOCAB, EMBED, COND = 100000, 64, 256
B, L = 16384, 50
NCORES = 8
BLOC = B // NCORES  # 2048 bags per core
P = 128
NT = BLOC // P  # 16 tiles per core
NWIN = 2
NZERO = 128  # zero rows per window, at rel 0..127 (just above the base) so
             # padding rel is POSITIVE — the SWDGE ucode drops trailing
             # NEGATIVE indices from the stream ("ignored at the end"), so
             # padding must never be negative. Spread over 128 distinct rows
             # so padding reads don't serialize on one HBM row.
WSPAN = 65536  # rows addressable per window (signed int16 around the base)
WBASE = [32768, 98304]  # signed dma_gather base row per window
VROWS0 = WSPAN - NZERO  # vocab ids in window 0
TROWS = VOCAB + NWIN * NZERO  # remapped table rows

F32 = mybir.dt.float32
I32 = mybir.dt.int32
I16 = mybir.dt.int16
KCHUNK = 8  # max gather columns per dma_gather (1024-index SWDGE ring limit)
NQ = 4  # SWDGE queues


def _chunks(kmat):
    """[(t, w, kw, goff_cols, ioff_cols)] per gather chunk + per-tile totals."""
    out = []
    idx_off = 0
    ktot = []
    for t in range(NT):
        goff = 0
        for w in range(NWIN):
            rem = kmat[t][w]
            while rem > 0:
                kw = min(rem, KCHUNK)
                out.append((t, w, kw, goff, idx_off))
                goff += kw
                idx_off += 8 * kw
                rem -= kw
        ktot.append(goff)
    return out, ktot, idx_off


def build_nc(kmat) -> bass.Bass:
    """kmat[t][w] = column budget for tile t, window w (same for all cores)."""
    chunks, ktot, idx_cols = _chunks(kmat)

    nc = bacc.Bacc("TRN2", target_bir_lowering=False, num_swdge_queues=NQ)

    idx = nc.declare_dram_parameter("idx", [P, idx_cols], I16, isOutput=False)
    aux = nc.declare_dram_parameter("aux", [P, NT * 3], F32, isOutput=False)
    emb = nc.declare_dram_parameter("emb", [TROWS, EMBED], F32, isOutput=False)
    wext = nc.declare_dram_parameter("wext", [EMBED + 2, COND], F32, isOutput=False)
    out = nc.declare_dram_parameter("out", [BLOC, COND], F32, isOutput=True)

    op = mybir.AluOpType

    with tile.TileContext(nc) as tc, ExitStack() as ctx:
        const = ctx.enter_context(tc.tile_pool(name="const", bufs=1))
        sb = ctx.enter_context(tc.tile_pool(name="sb", bufs=4))
        gp = ctx.enter_context(tc.tile_pool(name="gp", bufs=1))
        ps = ctx.enter_context(tc.tile_pool(name="ps", bufs=2, space="PSUM"))

        # One-time constants + all indices/aux upfront
        idt = const.tile([P, P], F32, tag="idt")
        make_identity(nc, idt[:])
        wext_sb = const.tile([EMBED + 2, COND], F32, tag="wext")
        nc.sync.dma_start(out=wext_sb[:], in_=wext[:])
        idx_sb = const.tile([P, idx_cols], I16, tag="idx")
        nc.sync.dma_start(out=idx_sb[:], in_=idx[:, :])
        aux_sb = const.tile([P, NT * 3], F32, tag="aux")
        nc.sync.dma_start(out=aux_sb[:], in_=aux[:, :])

        by_tile: dict = {}
        for t, w, kw, goff, ioff in chunks:
            by_tile.setdefault(t, []).append((w, kw, goff, ioff))

        q = 0
        for t in range(NT):
            kt = ktot[t]
            g_t = gp.tile([P, kt * EMBED], F32, tag=f"g{t % 5}")
            for w, kw, goff, ioff in by_tile[t]:
                gv = g_t[:, goff * EMBED : (goff + kw) * EMBED].rearrange(
                    "p (k e) -> p k e", k=kw, e=EMBED
                )
                nc.gpsimd.dma_gather(
                    gv,
                    emb[WBASE[w] :, :],
                    idx_sb[:, ioff : ioff + 8 * kw],
                    P * kw,
                    P * kw,
                    EMBED,
                    queue_num=q % NQ,
                )
                q += 1
            # Sum the kt columns by contiguous in-place halving adds —
            # far faster on the DVE than one strided-AP reduce.
            n = kt
            while n > 1:
                h = n // 2
                nc.vector.tensor_tensor(
                    out=g_t[:, 0 : h * EMBED],
                    in0=g_t[:, 0 : h * EMBED],
                    in1=g_t[:, (n - h) * EMBED : n * EMBED],
                    op=op.add,
                )
                n -= h

            # mean = sum * (1/max(len,1)); append the two select flags
            tr = sb.tile([P, EMBED + 2], F32, tag="tr")
            nc.vector.tensor_scalar_mul(
                out=tr[:, 0:EMBED],
                in0=g_t[:, 0:EMBED],
                scalar1=aux_sb[:, 3 * t : 3 * t + 1],
            )
            nc.vector.tensor_copy(
                out=tr[:, EMBED : EMBED + 2], in_=aux_sb[:, 3 * t + 1 : 3 * t + 3]
            )

            # [P, 66] -> [66, P] so the projection contracts over E on partitions
            pT = ps.tile([EMBED + 2, P], F32, tag="pT", space="PSUM")
            nc.tensor.transpose(out=pT[:], in_=tr[:], identity=idt[:])
            mT = sb.tile([EMBED + 2, P], F32, tag="mT")
            nc.scalar.copy(out=mT[:], in_=pT[:])

            # out[128, 256] = meanT.T @ [W.T; b; null]: proj + bias + null select
            po = ps.tile([P, COND], F32, tag="po", space="PSUM")
            nc.tensor.matmul(
                out=po[:], lhsT=mT[:], rhs=wext_sb[:], start=True, stop=True
            )
            ob = sb.tile([P, COND], F32, tag="ob")
            nc.scalar.copy(out=ob[:], in_=po[:])
            nc.scalar.dma_start(out=out[t * P : (t + 1) * P, :], in_=ob[:])

    nc.compile()
    return nc


_CACHE: dict = {}


def _prep(token_ids, lengths):
    """Sort bags by length per core, split tokens by vocab window, compute
    column budgets. Returns (kmat, per-core idx arrays, per-core aux, perms)."""
    ids_all = np.asarray(token_ids).astype(np.int64, copy=False)
    lens_all = np.asarray(lengths).astype(np.int64, copy=False)

    cores = []
    for c in range(NCORES):
        ids = ids_all[c * BLOC : (c + 1) * BLOC]
        lens = lens_all[c * BLOC : (c + 1) * BLOC]
        # remap: zero rows sit at rel 0..127 of each window's base
        new_ids = ids + np.where(ids < WBASE[0], 0, NZERO) + np.where(
            ids < WBASE[1] - NZERO, 0, NZERO
        )
        win = (ids >= VROWS0).astype(np.int64)  # 0 or 1
        rel0 = new_ids - np.take(np.array(WBASE), win)  # signed window-relative
        validv = np.arange(L)[None, :] < lens[:, None]
        # sort bags by (length desc, window-0 count desc) so tiles are
        # homogeneous in both → tight per-tile column budgets
        n0 = ((win == 0) & validv).sum(axis=1)
        n1 = ((win == 1) & validv).sum(axis=1)
        order = np.lexsort((-n1, -n0))
        lens = lens[order]
        win, rel = win[order], rel0[order]
        valid = validv[order]
        aux = np.stack(
            [
                1.0 / np.maximum(lens, 1),
                (lens > 0),
                (lens == 0),
            ],
            axis=1,
        ).astype(np.float32)
        cores.append((order, lens, win, rel, valid, aux))

    counts = np.zeros((NCORES, NT, NWIN, P), np.int64)
    nonneg127 = np.zeros((NCORES, NT, NWIN), np.int64)  # bag at partition 127
    for c, (_, _, win, rel, valid, _) in enumerate(cores):
        for w in range(NWIN):
            sel = (win == w) & valid
            cnt = sel.sum(axis=1)  # [BLOC]
            counts[c, :, w, :] = cnt.reshape(NT, P)
            nn = (sel & (rel >= 0)).sum(axis=1).reshape(NT, P)
            nonneg127[c, :, w] = nn[:, P - 1]
    kmat = counts.max(axis=(0, 3))  # [NT, NWIN] shared across cores
    # The ucode drops the stream's trailing run of NEGATIVE indices, so every
    # chunk's final stream entry (partition 127, last column) must be >= 0.
    # Row 127's entries are freely permutable; ensure it holds at least
    # ceil(K/KCHUNK) non-negative entries (padding counts) on every core.
    cnt127 = counts[:, :, :, P - 1]  # [NCORES, NT, NWIN]
    for t in range(NT):
        for w in range(NWIN):
            K = int(kmat[t][w])
            if K == 0:
                continue
            avail = int((K - cnt127[:, t, w] + nonneg127[:, t, w]).min())
            while avail < -(-K // KCHUNK):
                K += 1
                avail += 1
            kmat[t][w] = K
    for t in range(NT):
        if kmat[t].sum() == 0:
            kmat[t][0] = 1
    kmat = kmat.tolist()

    idx_arrs, aux_arrs, perms = [], [], []
    for c, (order, lens, win, rel, valid, aux) in enumerate(cores):
        blocks = []
        for t in range(NT):
            rows = slice(t * P, (t + 1) * P)
            winb, relb, validb = win[rows], rel[rows], valid[rows]
            for w in range(NWIN):
                kw = kmat[t][w]
                if kw == 0:
                    continue
                sel = (winb == w) & validb  # [P, L]
                cnt = sel.sum(axis=1)  # [P]
                pos = np.argsort(~sel, axis=1, kind="stable")[:, :kw]
                vals = np.take_along_axis(relb, pos, axis=1)
                colmask = np.arange(kw)[None, :] < cnt[:, None]
                padfill = (
                    np.arange(P)[:, None] + np.arange(kw)[None, :] * 17
                ) % NZERO  # positive rel: the window's zero rows
                padded = np.where(colmask, vals, padfill)  # [P, kw]
                # place non-negatives at every chunk-last column of row 127
                bounds = list(range(KCHUNK - 1, kw, KCHUNK))
                if (kw - 1) not in bounds:
                    bounds.append(kw - 1)
                row = padded[P - 1].copy()
                nn = row[row >= 0]
                ng = row[row < 0]
                assert len(nn) >= len(bounds), (t, w, len(nn), len(bounds))
                newrow = np.empty_like(row)
                others = [i for i in range(kw) if i not in bounds]
                newrow[bounds] = nn[: len(bounds)]
                rest = np.concatenate([nn[len(bounds) :], ng])
                newrow[others] = rest
                padded[P - 1] = newrow
                for c0 in range(0, kw, KCHUNK):
                    chunk = padded[:, c0 : c0 + KCHUNK]
                    flat = chunk.T.ravel()  # stream order i = col*128 + bag
                    blk = flat.reshape(-1, 16).T  # [16, P*kc/16]
                    blocks.append(np.tile(blk, (8, 1)))
        idx_arrs.append(
            np.ascontiguousarray(np.concatenate(blocks, axis=1).astype(np.int16))
        )
        # aux in [P, NT*3] layout: aux_sb[p, 3t+j] = aux[t*128+p, j]
        aux_arrs.append(
            np.ascontiguousarray(
                aux.reshape(NT, P, 3).transpose(1, 0, 2).reshape(P, NT * 3)
            )
        )
        perms.append(order)
    return kmat, idx_arrs, aux_arrs, perms


def make_in_maps(token_ids, lengths, emb_table, W, b, null_emb):
    kmat, idx_arrs, aux_arrs, perms = _prep(token_ids, lengths)

    emb_src = np.asarray(emb_table, dtype=np.float32)
    emb_ext = np.zeros((TROWS, EMBED), np.float32)
    ar = np.arange(VOCAB)
    new_rows = ar + np.where(ar < 32768, 0, NZERO) + np.where(
        ar < 98304 - NZERO, 0, NZERO
    )
    emb_ext[new_rows] = emb_src

    wext = np.concatenate(
        [
            np.asarray(W, dtype=np.float32).T,  # [64, 256]
            np.asarray(b, dtype=np.float32)[None, :],
            np.asarray(null_emb, dtype=np.float32)[None, :],
        ]
    )  # [66, 256]
    in_maps = [
        {
            "idx": idx_arrs[c],
            "aux": aux_arrs[c],
            "emb": emb_ext,
            "wext": wext,
        }
        for c in range(NCORES)
    ]
    return kmat, in_maps, perms


def kernel(token_ids, lengths, emb_table, W, b, null_emb, **run_kwargs):
    from concourse.bass_utils import run_bass_kernel_spmd

    kmat, in_maps, perms = make_in_maps(
        token_ids, lengths, emb_table, W, b, null_emb
    )
    key = tuple(tuple(kr) for kr in kmat)
    if key not in _CACHE:
        _CACHE[key] = build_nc(kmat)
    nc = _CACHE[key]
    res = run_bass_kernel_spmd(nc, in_maps, core_ids=list(range(NCORES)), **run_kwargs)
    global _LAST_RES
    _LAST_RES = res
    out = np.empty((B, COND), np.float32)
    for c in range(NCORES):
        out[c * BLOC + perms[c]] = res.results[c]["out"]
    return out
